# revision 1
# baseline (speedup 1.0000x reference)
"""Trainium2 Bass kernel for single-head causal attention (B=4, T=4096, C=2048, HS=128).

Sharding: 2 cores per batch element (8 cores, B=4). Each core owns 2048
sequence rows of its batch element, arranged (by the host) as an interleaved
"fold" of 512-row query tiles so causal attention work is balanced:
  role A (even cores): global q-tiles [7, 5, 2, 0]
  role B (odd cores):  global q-tiles [6, 4, 3, 1]
Each core projects Q^T/K^T/V^T for its own rows (contraction over C with W
stationary), AllGathers K^T/V^T within its pair, DMA-transposes V^T -> V,
then computes causal attention for its 4 q-tiles with a static per-slot
k-extent [32, 24, 16, 8] (k-tiles of 128). Role differences are expressed
purely through input data (row ordering + mask tiles), so all 8 cores run
one SPMD graph. Softmax skips the row-max (scores are O(1) by construction);
masking multiplies exp values by {0,1}/triangular tiles after the exp.
"""

import math

import numpy as np
import ml_dtypes

import concourse.bacc as bacc
import concourse.tile as tile
from concourse import mybir
from concourse.bass_utils import run_bass_kernel_spmd

B, T, C, HS = 4, 4096, 2048, 128
NCORES = 8
TOWN = 2048              # sequence rows owned per core
NCT = C // 128           # 16 contraction tiles
QTILES_A = [7, 5, 2, 0]  # global 512-row q-tiles per slot, role A
QTILES_B = [6, 4, 3, 1]  # role B
ES = [32, 24, 16, 8]     # static per-slot k extents (k-tiles of 128)
# global 512-row k-chunk -> (group position, slot) in the gathered KV buffer
CHUNK_ROLE = [0, 1, 0, 1, 1, 0, 1, 0]
CHUNK_SLOT = [3, 3, 2, 2, 1, 1, 0, 0]

BF16 = ml_dtypes.bfloat16


def build_graph():
    nc = bacc.Bacc(
        "TRN2", target_bir_lowering=False, debug=False, num_devices=NCORES
    )
    bf = mybir.dt.bfloat16
    f32 = mybir.dt.float32

    xt_d = nc.dram_tensor("xt", [128, NCT, TOWN], bf, kind="ExternalInput")
    wq_d = nc.dram_tensor("wq", [128, NCT, HS], bf, kind="ExternalInput")
    wk_d = nc.dram_tensor("wk", [128, NCT, HS], bf, kind="ExternalInput")
    wv_d = nc.dram_tensor("wv", [128, NCT, HS], bf, kind="ExternalInput")
    msk_d = nc.dram_tensor("msk", [128, 32, 512], bf, kind="ExternalInput")
    idn_d = nc.dram_tensor("idn", [128, 128], f32, kind="ExternalInput")
    onc_d = nc.dram_tensor("onc", [128, 1], bf, kind="ExternalInput")
    onr_d = nc.dram_tensor("onr", [1, 128], f32, kind="ExternalInput")
    out_d = nc.dram_tensor("out", [TOWN, HS], f32, kind="ExternalOutput")

    with tile.TileContext(nc) as tc:
        with (
            tc.tile_pool(name="big", bufs=1) as big,
            tc.tile_pool(name="dram", bufs=1, space="DRAM") as dram,
        ):
            xt = big.tile([128, NCT, TOWN], bf, tag="xt")
            nc.sync.dma_start(xt[:], xt_d[:])
            wq = big.tile([128, NCT, HS], bf, tag="wq")
            nc.sync.dma_start(wq[:], wq_d[:])
            wk = big.tile([128, NCT, HS], bf, tag="wk")
            nc.sync.dma_start(wk[:], wk_d[:])
            wv = big.tile([128, NCT, HS], bf, tag="wv")
            nc.sync.dma_start(wv[:], wv_d[:])
            msk = big.tile([128, 32, 512], bf, tag="msk")
            nc.sync.dma_start(msk[:], msk_d[:])
            idn = big.tile([128, 128], f32, tag="idn")
            nc.sync.dma_start(idn[:], idn_d[:])
            onc = big.tile([128, 1], bf, tag="onc")
            nc.sync.dma_start(onc[:], onc_d[:])
            onr = big.tile([1, 128], f32, tag="onr")
            nc.sync.dma_start(onr[:], onr_d[:])

            ktq = big.tile([128, TOWN], bf, tag="ktq")  # own K^T, slot-order cols
            vtq = big.tile([128, TOWN], bf, tag="vtq")  # own V^T
            qt = big.tile([128, TOWN], bf, tag="qt")    # own Q^T (pre-scaled)
            ktf = big.tile([128, T], bf, tag="ktf")     # gathered K^T, global order
            v3 = big.tile([128, T // 128, HS], bf, tag="v3")  # V k-tiles

            kvb = dram.tile([256, TOWN], bf, tag="kvb")
            kvg = dram.tile([512, TOWN], bf, tag="kvg")

            # ---- projections (K, V first so the AllGather can fire early) ----
            with tc.tile_pool(name="pjps", bufs=2, space="PSUM") as pjps:
                for w, dest in [(wk, ktq), (wv, vtq), (wq, qt)]:
                    for t4 in range(4):
                        ps = pjps.tile([128, 512], f32, tag="pj")
                        for c in range(NCT):
                            nc.tensor.matmul(
                                ps[:],
                                w[:, c, :],
                                xt[:, c, t4 * 512 : (t4 + 1) * 512],
                                start=(c == 0),
                                stop=(c == NCT - 1),
                            )
                        nc.vector.tensor_copy(
                            dest[:, t4 * 512 : (t4 + 1) * 512], ps[:]
                        )
                    if dest is ktq:
                        nc.sync.dma_start(kvb[0:128, :], ktq[:])
                    elif dest is vtq:
                        nc.sync.dma_start(kvb[128:256, :], vtq[:])
                        nc.gpsimd.collective_compute(
                            "AllGather",
                            mybir.AluOpType.bypass,
                            replica_groups=[[0, 1], [2, 3], [4, 5], [6, 7]],
                            ins=[kvb.opt()],
                            outs=[kvg.opt()],
                        )

                # reorder gathered KV into global k order
                for g in range(8):
                    base = CHUNK_ROLE[g] * 256
                    s = CHUNK_SLOT[g]
                    nc.sync.dma_start(
                        ktf[:, g * 512 : (g + 1) * 512],
                        kvg[base : base + 128, s * 512 : (s + 1) * 512],
                    )
                    nc.sync.dma_start_transpose(
                        v3[:, g * 4 : (g + 1) * 4, :],
                        kvg[base + 128 : base + 256, s * 512 : (s + 1) * 512],
                    )

            # ---- attention ----
            with (
                tc.tile_pool(name="ops", bufs=1, space="PSUM") as ops,
                tc.tile_pool(name="dps", bufs=2, space="PSUM") as dps,
                tc.tile_pool(name="sps", bufs=2, space="PSUM") as sps,
                tc.tile_pool(name="tps", bufs=1, space="PSUM") as tps,
                tc.tile_pool(name="pp", bufs=3) as pp,
                tc.tile_pool(name="ep", bufs=2) as ep,
                tc.tile_pool(name="yp", bufs=3) as yp,
            ):
                for s in range(4):
                    E = ES[s]
                    o_ps = ops.tile([128, 512], f32, tag="o")
                    d_ps = dps.tile([1, 512], f32, tag="d")
                    for k2 in range(E // 2):
                        s_ps = sps.tile([128, 1024], f32, tag="s")
                        for hf in range(2):
                            kb = k2 * 2 + hf
                            nc.tensor.matmul(
                                s_ps[:, hf * 512 : (hf + 1) * 512],
                                ktf[:, kb * 128 : (kb + 1) * 128],
                                qt[:, s * 512 : (s + 1) * 512],
                                start=True,
                                stop=True,
                            )
                        p_sb = pp.tile([128, 1024], bf, tag="p")
                        nc.scalar.activation(
                            p_sb[:], s_ps[:], mybir.ActivationFunctionType.Exp
                        )
                        for hf in range(2):
                            kb = k2 * 2 + hf
                            j = kb - (E - 8)
                            ph = p_sb[:, hf * 512 : (hf + 1) * 512]
                            if j >= 0:
                                nc.vector.tensor_mul(
                                    ph, ph, msk[:, s * 8 + j, :]
                                )
                            nc.tensor.matmul(
                                o_ps[:],
                                v3[:, kb, :],
                                ph,
                                start=(kb == 0),
                                stop=(kb == E - 1),
                                skip_group_check=True,
                            )
                            nc.tensor.matmul(
                                d_ps[:],
                                onc[:],
                                ph,
                                start=(kb == 0),
                                stop=(kb == E - 1),
                                skip_group_check=True,
                            )
                    # epilogue: normalize + transpose out
                    ot = ep.tile([128, 512], f32, tag="ot")
                    nc.vector.tensor_copy(ot[:], o_ps[:])
                    den = ep.tile([1, 512], f32, tag="den")
                    nc.vector.tensor_copy(den[:], d_ps[:])
                    rec = ep.tile([1, 512], f32, tag="rec")
                    nc.vector.reciprocal(rec[:], den[:])
                    rb = dps.tile([128, 512], f32, tag="d")
                    nc.tensor.matmul(rb[:], onr[:], rec[:], start=True, stop=True)
                    nc.vector.tensor_mul(ot[:], ot[:], rb[:])
                    for u in range(4):
                        tp = tps.tile([128, 128], f32, tag="tp")
                        nc.tensor.transpose(
                            tp[:], ot[:, u * 128 : (u + 1) * 128], idn[:]
                        )
                        y = yp.tile([128, 128], f32, tag="y")
                        nc.vector.tensor_copy(y[:], tp[:])
                        nc.gpsimd.dma_start(
                            out_d[s * 512 + u * 128 : s * 512 + (u + 1) * 128, :],
                            y[:],
                        )

    nc.compile()
    return nc


def _role_qtiles(h):
    return QTILES_A if h == 0 else QTILES_B


def _make_masks(h):
    """[128, 32, 512] bf16 mask tiles, indexed [k, s*8+j, q]."""
    qtiles = _role_qtiles(h)
    eact = [4 * (g + 1) for g in qtiles]
    m = np.zeros((128, 32, 512), np.float32)
    k = np.arange(128)[:, None]
    q = np.arange(512)[None, :]
    for s in range(4):
        E = ES[s]
        for j in range(8):
            kb = E - 8 + j
            if kb < eact[s] - 4:
                m[:, s * 8 + j, :] = 1.0
            elif kb < eact[s]:
                d = kb - (eact[s] - 4)
                m[:, s * 8 + j, :] = (128 * d + k <= q).astype(np.float32)
            # else: stays zero
    return m.astype(BF16)


def make_in_maps(x, Wq, Wk, Wv):
    """Host-side sharding + layout prep. x [B,T,C] f32, W* [C,HS] f32."""
    wq_s = (np.asarray(Wq, np.float32) / math.sqrt(HS))
    w_arr = {}
    for nm, w in [("wq", wq_s), ("wk", np.asarray(Wk)), ("wv", np.asarray(Wv))]:
        # [C, HS] -> [128, NCT, HS] with row c = ci*128 + p
        w_arr[nm] = np.ascontiguousarray(
            w.reshape(NCT, 128, HS).transpose(1, 0, 2)
        ).astype(BF16)
    idn = np.eye(128, dtype=np.float32)
    onc = np.ones((128, 1), np.float32).astype(BF16)
    onr = np.ones((1, 128), np.float32)
    msk_by_role = [_make_masks(0), _make_masks(1)]

    in_maps = []
    for core in range(NCORES):
        b, h = core // 2, core % 2
        qtiles = _role_qtiles(h)
        rows = np.concatenate(
            [np.arange(g * 512, (g + 1) * 512) for g in qtiles]
        )
        xr = np.asarray(x[b])[rows]  # [2048 rows, C] f32
        xT = np.ascontiguousarray(xr.T).astype(BF16)  # [C, 2048]
        xt_arr = np.ascontiguousarray(
            xT.reshape(NCT, 128, TOWN).transpose(1, 0, 2)
        )  # [128, NCT, 2048]
        in_maps.append(
            {
                "xt": xt_arr,
                "wq": w_arr["wq"],
                "wk": w_arr["wk"],
                "wv": w_arr["wv"],
                "msk": msk_by_role[h],
                "idn": idn,
                "onc": onc,
                "onr": onr,
            }
        )
    return in_maps


def assemble_out(results):
    """results: list of 8 dicts with 'out' [2048, 128] -> y [B,T,HS] f32."""
    y = np.zeros((B, T, HS), np.float32)
    for core in range(NCORES):
        b, h = core // 2, core % 2
        qtiles = _role_qtiles(h)
        o = np.asarray(results[core]["out"])
        for s in range(4):
            g = qtiles[s]
            y[b, g * 512 : (g + 1) * 512] = o[s * 512 : (s + 1) * 512]
    return y


_NC_CACHE = None


def _get_graph():
    global _NC_CACHE
    if _NC_CACHE is None:
        _NC_CACHE = build_graph()
    return _NC_CACHE


def kernel(x, Wq, Wk, Wv):
    nc = _get_graph()
    in_maps = make_in_maps(x, Wq, Wk, Wv)
    res = run_bass_kernel_spmd(nc, in_maps, list(range(NCORES)))
    return assemble_out(res.results)
